# revision 11
# baseline (speedup 1.0000x reference)
"""Trainium2 Bass kernel for a transformer attention block (BasicBlock).

Reference computation (B=2, L=2048, D=1024, H=16, C=64):
    qkv = x @ w_qkv.T + b_qkv ; q,k,v = split(qkv)
    attn = softmax((q @ k.T) / sqrt(D)) ; heads = attn @ v
    out  = heads @ w_o.T + b_o + x

Sharding: 8 cores = 2 batches x 4 head-groups (4 heads each).
Each core computes, for its (b, g):
    qkT = (w_qk_g @ x_b.T) + b_qk_g          [512, 2048]  (Q^T, K^T per head)
    V   = x_b @ w_v_g.T                      [2048, 256]  (bias folded on host)
    S^T_h = K_h^T.T-free scores via zero-padded K=128 matmuls
    P^T = exp(S^T * scale)  (no max-subtraction; scores are small by construction)
    O'^T_h = [V_h | 1]^T @ P^T  -> rows 0..63 = O^T, row 64 = softmax denominators
    O^T_h normalized via broadcasted reciprocal of row 64
    partial = O @ w_o[:, cols_g].T           [2048, 1024]
Host sums the 4 group partials per batch and adds x + b_o + w_o @ b_v.
"""

import sys

if "/opt/trn_rl_repo" not in sys.path:
    sys.path.insert(0, "/opt/trn_rl_repo")

import numpy as np

B, L, D, H = 2, 2048, 1024, 16
C = 64
HPC = 4            # heads per core
G = 256            # dims per head group (HPC * C)
SCALE = float(1.0 / np.sqrt(np.float32(D)))

LC = 512           # l-chunk (moving dim)
NLC = L // LC      # 4
MT = L // 128      # 16 m-tiles
DT = D // 128      # 8 d-tiles
NEC = D // 512     # 2 e-chunks for out projection

_CACHE = {}


def _build(reps=1):
    import concourse.bass as bass
    import concourse.mybir as mybir
    import concourse.tile as tile
    from concourse import bacc

    f32 = mybir.dt.float32
    f32r = mybir.dt.float32r
    Exp = mybir.ActivationFunctionType.Copy  # placeholder, set below
    Exp = mybir.ActivationFunctionType.Exp
    Copy = mybir.ActivationFunctionType.Copy

    nc = bacc.Bacc("TRN2", target_bir_lowering=False, debug=False)

    xT = nc.declare_dram_parameter("xT", [D, L], f32r, isOutput=False)
    wqkT = nc.declare_dram_parameter("wqkT", [D, 2 * G], f32r, isOutput=False)
    bqk = nc.declare_dram_parameter("bqk", [128, 4], f32, isOutput=False)
    wvT = nc.declare_dram_parameter("wvT", [D, G], f32r, isOutput=False)
    woT = nc.declare_dram_parameter("woT", [G, D], f32r, isOutput=False)
    out = nc.declare_dram_parameter("out", [L, D], f32, isOutput=True)

    def r(ap):
        return ap.bitcast(f32r)

    with tile.TileContext(nc) as tc:
      for _rep in range(reps):
        with (
            tc.tile_pool(name="const", bufs=1) as constp,
            tc.tile_pool(name="qp", bufs=2) as qpp,
            tc.tile_pool(name="kz", bufs=4) as kzp,
            tc.tile_pool(name="vt", bufs=16) as vtp,
            tc.tile_pool(name="wo", bufs=2) as wop,
            tc.tile_pool(name="ot", bufs=2) as otp,
            tc.tile_pool(name="ps_mm", bufs=2, space="PSUM") as psmm,
            tc.tile_pool(name="ps_sc", bufs=2, space="PSUM") as pssc,
            tc.tile_pool(name="ps_o", bufs=2, space="PSUM") as pso,
        ):
            # ---- constants / weights that live the whole kernel ----
            bqk_sb = constp.tile([128, 4], f32)
            nc.sync.dma_start(out=bqk_sb[:], in_=bqk[:])

            wo_sb = []
            for t in range(2):
                w = wop.tile([128, D], f32r)
                nc.sync.dma_start(out=w[:], in_=woT[t * 128:(t + 1) * 128, :])
                wo_sb.append(w)

            # qp[p]: Q^T head pair tiles: partitions 0-63 = head 2p, 64-127 = head 2p+1
            qp = [qpp.tile([128, L], f32r, name="qp", tag="qp") for _ in range(2)]
            # kz[h]: K^T_h zero-padded to 128 partitions (head at its parity offset)
            kz = [kzp.tile([128, L], f32r, name="kz", tag="kz") for h in range(HPC)]

            # v[mt]: [128, 260]; per head block of 65 cols: [V_h (64) | ones]
            vt = [vtp.tile([128, HPC * 65], f32r, name="vt", tag="vt") for _ in range(MT)]

            ot = [otp.tile([128, L], f32r, name="ot", tag="ot") for _ in range(2)]

            with (
                tc.tile_pool(name="xt", bufs=DT) as xtp,
                tc.tile_pool(name="wqk", bufs=DT) as wqkp,
                tc.tile_pool(name="wv", bufs=DT) as wvp,
            ):
                xt, wqk, wv = [], [], []
                for i in range(DT):
                    x_sb = xtp.tile([128, L], f32r)
                    for c in range(NLC):
                        cs = slice(c * LC, (c + 1) * LC)
                        nc.sync.dma_start(out=x_sb[:, cs], in_=xT[i * 128:(i + 1) * 128, cs])
                    xt.append(x_sb)
                    w = wqkp.tile([128, 2 * G], f32r, name="wqk_sb", tag="wqk_sb")
                    nc.sync.dma_start(out=w[:], in_=wqkT[i * 128:(i + 1) * 128, :])
                    wqk.append(w)
                    w2 = wvp.tile([128, G], f32r, name="wv_sb", tag="wv_sb")
                    nc.sync.dma_start(out=w2[:], in_=wvT[i * 128:(i + 1) * 128, :])
                    wv.append(w2)
                # zero the unused head-parity halves of kz (memset cannot
                # write f32r; multiply loaded data by 0 instead)
                for h in range(HPC):
                    zs = slice(64, 128) if h % 2 == 0 else slice(0, 64)
                    nc.gpsimd.tensor_scalar_mul(kz[h][zs, :], xt[0][zs, :], 0.0)

                # ---- P1: qkT = wqk^T.T @ xT  (+bias at eviction) ----
                # K tiles first (t=2: heads 0/1, t=3: heads 2/3) so the first
                # attention units can start while Q of later heads computes.
                for t in [2, 0, 3, 1]:
                    for lc in range(NLC):
                        ps = psmm.tile([128, LC], f32)
                        for d in range(DT):
                            nc.tensor.matmul(
                                ps[:],
                                lhsT=r(wqk[d][:, t * 128:(t + 1) * 128]),
                                rhs=r(xt[d][:, lc * LC:(lc + 1) * LC]),
                                start=(d == 0),
                                stop=(d == DT - 1),
                            )
                        ls = slice(lc * LC, (lc + 1) * LC)
                        if t < 2:
                            nc.vector.tensor_scalar_add(
                                qp[t][:, ls], ps[:], bqk_sb[:, t:t + 1]
                            )
                        else:
                            h0 = 2 * (t - 2)
                            nc.vector.tensor_scalar_add(
                                kz[h0][0:64, ls], ps[0:64, :], bqk_sb[0:64, t:t + 1]
                            )
                            nc.vector.tensor_scalar_add(
                                kz[h0 + 1][64:128, ls], ps[64:128, :],
                                bqk_sb[64:128, t:t + 1],
                            )

                # ---- P2: V = xT.T @ wvT  (no bias; folded on host) ----
                for mt in range(MT):
                    ps = psmm.tile([128, G], f32)
                    for d in range(DT):
                        nc.tensor.matmul(
                            ps[:],
                            lhsT=r(xt[d][:, mt * 128:(mt + 1) * 128]),
                            rhs=r(wv[d][:]),
                            start=(d == 0),
                            stop=(d == DT - 1),
                        )
                    v3d = vt[mt][:].rearrange("p (h c) -> p h c", h=HPC)
                    nc.vector.tensor_copy(
                        v3d[:, :, 0:64], ps[:].rearrange("p (h c) -> p h c", h=HPC)
                    )
                    nc.vector.tensor_scalar(
                        v3d[:, :, 64:65], v3d[:, :, 0:1], 0.0, 1.0,
                        mybir.AluOpType.mult, mybir.AluOpType.add,
                    )

            # ---- P3/P4 pools allocated after xt/wqk/wv free their SBUF ----
            from contextlib import ExitStack
            _p34 = ExitStack()
            ptp = _p34.enter_context(tc.tile_pool(name="pt", bufs=12))
            rcpp = _p34.enter_context(tc.tile_pool(name="rcp", bufs=3))
            nrmp = _p34.enter_context(tc.tile_pool(name="nrm", bufs=3))
            stgp = _p34.enter_context(tc.tile_pool(name="stg", bufs=4))

            # ---- P3: attention per (head, l-chunk) ----
            for h in range(HPC):
                po_off = (h % 2) * 64
                for lc in range(NLC):
                    ls = slice(lc * LC, (lc + 1) * LC)
                    pts = []
                    for j in range(MT // 2):
                        ps = pssc.tile([128, 2 * LC], f32)
                        for half in range(2):
                            mt = 2 * j + half
                            nc.tensor.matmul(
                                ps[:, half * LC:(half + 1) * LC],
                                lhsT=r(kz[h][:, mt * 128:(mt + 1) * 128]),
                                rhs=r(qp[h // 2][:, ls]),
                                start=True,
                                stop=True,
                            )
                        ptile = ptp.tile([128, 2 * LC], f32r)
                        nc.scalar.activation(ptile[:], ps[:], Exp, scale=SCALE)
                        pts.append(ptile)

                    po = pso.tile([65, LC], f32)
                    for j in range(MT // 2):
                        for half in range(2):
                            mt = 2 * j + half
                            nc.tensor.matmul(
                                po[:],
                                lhsT=r(vt[mt][:, h * 65:(h + 1) * 65]),
                                rhs=r(pts[j][:, half * LC:(half + 1) * LC]),
                                start=(mt == 0),
                                stop=(mt == MT - 1),
                            )

                    # normalize: rows 0..63 are O^T, row 64 is the denominator
                    rc = rcpp.tile([128, LC], f32)
                    nc.vector.reciprocal(rc[64:65, :], po[64:65, :])
                    # partition_broadcast reads physical partition 0 on HW;
                    # stage the reciprocal row there via a small SBUF DMA
                    rc0 = rcpp.tile([1, LC], f32, name="rc0")
                    nc.sync.dma_start(out=rc0[0:1, :], in_=rc[64:65, :])
                    rb = rcpp.tile([64, LC], f32)
                    nc.gpsimd.partition_broadcast(rb[:], rc0[0:1, :])
                    nt = nrmp.tile([64, LC], f32r)
                    nc.vector.tensor_mul(nt[:], po[0:64, :], rb[:])
                    # place at the head's parity offset in ot via SBUF->SBUF DMA
                    nc.sync.dma_start(
                        out=ot[h // 2][po_off:po_off + 64, ls], in_=nt[:]
                    )

            # ---- P4: out projection: out = O @ woT ----
            for lt in range(MT):
                for ec in range(NEC):
                    ps = psmm.tile([128, 512], f32)
                    for t in range(2):
                        nc.tensor.matmul(
                            ps[:],
                            lhsT=r(ot[t][:, lt * 128:(lt + 1) * 128]),
                            rhs=r(wo_sb[t][:, ec * 512:(ec + 1) * 512]),
                            start=(t == 0),
                            stop=(t == 1),
                        )
                    st = stgp.tile([128, 512], f32)
                    nc.vector.tensor_copy(st[:], ps[:])
                    nc.sync.dma_start(
                        out=out[lt * 128:(lt + 1) * 128, ec * 512:(ec + 1) * 512],
                        in_=st[:],
                    )
            _p34.close()

    nc.compile()
    return nc


def _prep_in_maps(x, w_qkv, b_qkv, w_o):
    xT = [np.ascontiguousarray(x[b].T) for b in range(B)]
    in_maps = []
    for core in range(8):
        b, g = divmod(core, 4)
        qs, ks, vs = g * G, D + g * G, 2 * D + g * G
        wqkT = np.ascontiguousarray(
            np.concatenate([w_qkv[qs:qs + G], w_qkv[ks:ks + G]], axis=0).T
        )
        bqk_m = np.ascontiguousarray(
            np.concatenate([b_qkv[qs:qs + G], b_qkv[ks:ks + G]]).reshape(4, 128).T
        )
        wvT = np.ascontiguousarray(w_qkv[vs:vs + G].T)
        woT = np.ascontiguousarray(w_o[:, g * G:(g + 1) * G].T)
        in_maps.append(
            {
                "xT": xT[b],
                "wqkT": wqkT,
                "bqk": bqk_m,
                "wvT": wvT,
                "woT": woT,
            }
        )
    return in_maps


def kernel(x, w_qkv, b_qkv, w_o, b_o):
    from concourse.bass_utils import run_bass_kernel_spmd

    x = np.asarray(x, dtype=np.float32)
    w_qkv = np.asarray(w_qkv, dtype=np.float32)
    b_qkv = np.asarray(b_qkv, dtype=np.float32)
    w_o = np.asarray(w_o, dtype=np.float32)
    b_o = np.asarray(b_o, dtype=np.float32)

    if "nc" not in _CACHE:
        _CACHE["nc"] = _build()
    nc = _CACHE["nc"]

    in_maps = _prep_in_maps(x, w_qkv, b_qkv, w_o)
    res = run_bass_kernel_spmd(nc, in_maps, list(range(8)))
    partial = np.stack([res.results[i]["out"] for i in range(8)])  # [8, L, D]

    const = w_o @ b_qkv[2 * D:] + b_o  # [D]
    out = partial.reshape(B, 4, L, D).sum(axis=1) + x + const[None, None, :]
    return out.astype(np.float32)
